# revision 16
# baseline (speedup 1.0000x reference)
"""Trainium2 Bass kernel for ColorEntropyLoss (v5).

Math (per batch b, attention map s):
    cd[b,s,c] = sum_h attn[b,s,h] * (grid[b,h] == c)     # 10-bin weighted histogram
    p = cd / (sum_c cd + 1e-8);  entropy = -sum_c p*ln(p+1e-8);  out = mean

Sharding: pure data parallelism over batch B=512 across 8 NeuronCores
(64 batches/core, 8 groups of 8 batches; a group packs 128 SBUF
partitions as 8 batches x 16 maps). The entropy itself (10 values per
(b,s) row after the histogram) is computed host-side from the DMA'd
histograms -- the "cheap all-reduce" of the sharding hint, slightly
fattened (164 KB/core) so that no on-chip engine serializes behind the
matmul stream.

Device-side layout strategy:
- Host pre-transposes attn to [pix, (b,s)] order and pre-casts to fp8e4
  (rel err ~1e-6 on the final mean, measured), so the kernel does ZERO
  on-chip transposes and reads only 4.2 MB/core of HBM.
- grids are uploaded pre-transposed as bf16 [pix_in_chunk, (g,k,b)],
  group-major, and all input DMAs are plain HWDGE on the SP ring in the
  order grid0 -> attn0 -> grid1 -> attn1..7 so the small grid halves
  that gate the one-hot builds are never starved behind the attn stream.
- Histogram: 32 PSUM-accumulated matmuls per group,
    ps[(b,s),(c,b')] += attnT_chunk[128pix,128(b,s)].T @ onehot[128pix,80(c,b')]
  fp8 stationary x bf16 moving. One is_equal per group builds the
  one-hot masks on DVE (~1.4us each), pipelined one group ahead of the
  PE; the DMA stream (~1.5us/group) is the pacing pole.
- Per group the only post-op is an ACT psum->SBUF Copy (bf16); cd ~200
  so bf16's 0.4% rounding is mean-zero noise ~1e-5 on the final mean.
"""

import numpy as np
from contextlib import ExitStack

NUM_COLORS = 10
EPS = 1e-8
B, S, H, W = 512, 16, 64, 64
HW = H * W                      # 4096
N_CORES = 8
B_PER_CORE = B // N_CORES       # 64
N_GROUPS = 8                    # groups per core
B_PER_GROUP = B_PER_CORE // N_GROUPS  # 8 batches -> 128 partitions
P = 128
CHUNK = 128
N_CHUNKS = HW // CHUNK          # 32
NC80 = B_PER_GROUP * NUM_COLORS  # 80
GRID_COLS = N_GROUPS * N_CHUNKS * B_PER_GROUP  # 2048, col = g*256 + k*8 + b

USE_FP8 = True

_CACHE = {}


def _build_nc():
    import concourse.bacc as bacc
    import concourse.tile as tile
    import concourse.bass as bass
    from concourse import mybir

    f32 = mybir.dt.float32
    bf16 = mybir.dt.bfloat16
    attn_dt = mybir.dt.float8e4 if USE_FP8 else bf16
    OP = mybir.AluOpType
    AF = mybir.ActivationFunctionType

    nc = bacc.Bacc(
        "TRN2",
        target_bir_lowering=False,
        debug=False,
        num_devices=N_CORES,
        enable_partition_id=False,
    )

    attn_in = nc.dram_tensor(
        "attn_in", [N_GROUPS * P, HW], attn_dt, kind="ExternalInput"
    ).ap()
    grid_in = nc.dram_tensor(
        "grid_in", [P, GRID_COLS], bf16, kind="ExternalInput"
    ).ap()
    # raw 80-col histograms per group; host extracts the block diagonal
    cd_out = nc.dram_tensor(
        "cd_out", [P, N_GROUPS * NC80], bf16, kind="ExternalOutput"
    ).ap()

    with tile.TileContext(nc) as tc:
        with ExitStack() as ctx:
            singles = ctx.enter_context(tc.tile_pool(name="singles", bufs=1))
            psum = ctx.enter_context(
                tc.tile_pool(name="psum", bufs=4, space="PSUM")
            )

            # const_cb[p, c*8+b] = c (0..9, exact in bf16) -- needed by the
            # first mask op; Pool is free.
            const_cb = singles.tile([P, NC80], bf16)
            nc.gpsimd.iota(
                const_cb,
                pattern=[[1, NUM_COLORS], [0, B_PER_GROUP]],
                base=0,
                channel_multiplier=0,
                allow_small_or_imprecise_dtypes=True,
            )

            # Strictly ordered HWDGE DMAs on the SP ring. Host grid layout
            # col = g*256 + k*8 + b: the first half covers groups 0-3 and
            # gates mask0; the second half rides behind the first attn group.
            gridT = singles.tile([P, GRID_COLS], bf16)
            half = GRID_COLS // 2
            nc.sync.dma_start(out=gridT[:, 0:half], in_=grid_in[:, 0:half])

            attn_sb = []
            for g in range(N_GROUPS):
                t = singles.tile([P, HW], attn_dt, name=f"attn{g}")
                if g == N_GROUPS - 1:
                    # split the last group so the PE tail trails the DMA
                    # stream by only half a group
                    hw2 = HW // 2
                    nc.sync.dma_start(
                        out=t[:, 0:hw2], in_=attn_in[g * P : (g + 1) * P, 0:hw2]
                    )
                    nc.sync.dma_start(
                        out=t[:, hw2:HW], in_=attn_in[g * P : (g + 1) * P, hw2:HW]
                    )
                else:
                    nc.sync.dma_start(out=t, in_=attn_in[g * P : (g + 1) * P, :])
                attn_sb.append(t)
                if g == 0:
                    nc.sync.dma_start(
                        out=gridT[:, half:GRID_COLS],
                        in_=grid_in[:, half:GRID_COLS],
                    )

            cdq_all = singles.tile([P, N_GROUPS * NC80], bf16)

            # ---- one-hot masks, one is_equal per group on DVE ----
            # mask_g flat [128, 2560]: col = k*80 + c*8 + b
            def build_mask(g):
                mq = singles.tile([P, N_CHUNKS * NC80], bf16, name=f"mask{g}")
                gT = gridT[:, :]
                in0 = bass.AP(
                    tensor=gT.tensor,
                    offset=gT.offset + g * (N_CHUNKS * B_PER_GROUP),
                    ap=[
                        gT.ap[0],
                        [B_PER_GROUP, N_CHUNKS],
                        [0, NUM_COLORS],
                        [1, B_PER_GROUP],
                    ],
                )
                cC = const_cb[:, :]
                in1 = bass.AP(
                    tensor=cC.tensor,
                    offset=cC.offset,
                    ap=[cC.ap[0], [0, N_CHUNKS], [1, NC80]],
                )
                mk = mq[:, :]
                mout = bass.AP(
                    tensor=mk.tensor,
                    offset=mk.offset,
                    ap=[mk.ap[0], [NC80, N_CHUNKS], [1, NC80]],
                )
                nc.vector.tensor_tensor(out=mout, in0=in0, in1=in1, op=OP.is_equal)
                return mq

            masks = [build_mask(0), build_mask(1)]

            for g in range(N_GROUPS):
                attn_bf = attn_sb[g]
                mask = masks[g]

                ps = psum.tile([P, 512], f32, name="ps", tag="ps")
                ps_c = ps[:, 0:NC80]
                for k in range(N_CHUNKS):
                    nc.tensor.matmul(
                        ps_c,
                        attn_bf[:, k * CHUNK : (k + 1) * CHUNK],
                        mask[:, k * NC80 : (k + 1) * NC80],
                        start=(k == 0),
                        stop=(k == N_CHUNKS - 1),
                    )

                if g + 2 < N_GROUPS:
                    masks.append(build_mask(g + 2))

                # the only post-op: PSUM -> SBUF bf16 on the idle ACT engine
                nc.scalar.activation(
                    cdq_all[:, g * NC80 : (g + 1) * NC80], ps_c, AF.Copy
                )
                if g == N_GROUPS - 2:
                    # ship groups 0-6 while group 7 computes; only the last
                    # 80 columns remain on the critical tail
                    cut = (N_GROUPS - 1) * NC80
                    nc.sync.dma_start(
                        out=cd_out[:, 0:cut], in_=cdq_all[:, 0:cut]
                    )

            cut = (N_GROUPS - 1) * NC80
            nc.sync.dma_start(
                out=cd_out[:, cut : N_GROUPS * NC80],
                in_=cdq_all[:, cut : N_GROUPS * NC80],
            )

    nc.compile()
    return nc


def _get_nc():
    if "nc" not in _CACHE:
        _CACHE["nc"] = _build_nc()
    return _CACHE["nc"]


def _make_in_maps(attn_weights, grids):
    import ml_dtypes

    attn_np_dt = ml_dtypes.float8_e4m3 if USE_FP8 else ml_dtypes.bfloat16
    attn = np.asarray(attn_weights, dtype=np.float32)   # [512,16,64,64]
    grid = np.asarray(grids)                            # [512,64,64]
    in_maps = []
    for c in range(N_CORES):
        lo = c * B_PER_CORE
        # (g,b,s,k,p) -> (g,p,k,b,s): row g*128+p, col k*128 + b*16 + s
        a = attn[lo : lo + B_PER_CORE].reshape(
            N_GROUPS, B_PER_GROUP, S, N_CHUNKS, CHUNK
        )
        a = np.ascontiguousarray(a.transpose(0, 4, 3, 1, 2)).reshape(
            N_GROUPS * CHUNK, HW
        )
        # (g,b,k,p) -> (p,g,k,b): col g*256 + k*8 + b
        g_ = grid[lo : lo + B_PER_CORE].reshape(
            N_GROUPS, B_PER_GROUP, N_CHUNKS, CHUNK
        )
        g_ = np.ascontiguousarray(g_.transpose(3, 0, 2, 1)).reshape(
            CHUNK, GRID_COLS
        )
        in_maps.append(
            {
                "attn_in": a.astype(attn_np_dt),
                "grid_in": g_.astype(np.float32).astype(ml_dtypes.bfloat16),
            }
        )
    return in_maps


def kernel(attn_weights: np.ndarray, grids: np.ndarray) -> np.ndarray:
    from concourse.bass_utils import run_bass_kernel_spmd

    nc = _get_nc()
    in_maps = _make_in_maps(attn_weights, grids)
    res = run_bass_kernel_spmd(nc, in_maps, core_ids=list(range(N_CORES)))

    # Host finale: pick the block diagonal (row p=(b,s) owns batch p//16
    # of its group), then entropy + mean in f64.
    b_idx = (np.arange(P) // S)[:, None, None, None]     # [128,1,1,1]
    total = 0.0
    for c in range(N_CORES):
        r = res.results[c]["cd_out"].astype(np.float64)  # [128, 640]
        r = r.reshape(P, N_GROUPS, NUM_COLORS, B_PER_GROUP)
        cd = np.take_along_axis(r, b_idx, axis=3)[..., 0]  # [128, 8, 10]
        s_ = cd.sum(-1) + EPS
        p_ = cd / s_[..., None]
        ent = -(p_ * np.log(p_ + EPS)).sum(-1)
        total += float(ent.sum())
    return np.float32(total / (B * S))


# revision 20
# speedup vs baseline: 1.0090x; 1.0090x over previous
"""Trainium2 Bass kernel for ColorEntropyLoss (v5).

Math (per batch b, attention map s):
    cd[b,s,c] = sum_h attn[b,s,h] * (grid[b,h] == c)     # 10-bin weighted histogram
    p = cd / (sum_c cd + 1e-8);  entropy = -sum_c p*ln(p+1e-8);  out = mean

Sharding: pure data parallelism over batch B=512 across 8 NeuronCores
(64 batches/core, 8 groups of 8 batches; a group packs 128 SBUF
partitions as 8 batches x 16 maps). The entropy itself (10 values per
(b,s) row after the histogram) is computed host-side from the DMA'd
histograms -- the "cheap all-reduce" of the sharding hint, slightly
fattened (164 KB/core) so that no on-chip engine serializes behind the
matmul stream.

Device-side layout strategy:
- Host pre-transposes attn to [pix, (b,s)] order and pre-casts to fp8e4
  (rel err ~1e-6 on the final mean, measured), so the kernel does ZERO
  on-chip transposes and reads only 4.2 MB/core of HBM.
- grids are uploaded pre-transposed as bf16 [pix_in_chunk, (g,k,b)],
  group-major, and all input DMAs are plain HWDGE on the SP ring in the
  order grid0 -> attn0 -> grid1 -> attn1..7 so the small grid halves
  that gate the one-hot builds are never starved behind the attn stream.
- Histogram: 32 PSUM-accumulated matmuls per group,
    ps[(b,s),(c,b')] += attnT_chunk[128pix,128(b,s)].T @ onehot[128pix,80(c,b')]
  fp8 stationary x bf16 moving. One is_equal per group builds the
  one-hot masks on DVE (~1.4us each), pipelined one group ahead of the
  PE; the DMA stream (~1.5us/group) is the pacing pole.
- Per group the only post-op is an ACT psum->SBUF Copy (bf16); cd ~200
  so bf16's 0.4% rounding is mean-zero noise ~1e-5 on the final mean.
"""

import numpy as np
from contextlib import ExitStack

NUM_COLORS = 10
EPS = 1e-8
B, S, H, W = 512, 16, 64, 64
HW = H * W                      # 4096
N_CORES = 8
B_PER_CORE = B // N_CORES       # 64
N_GROUPS = 8                    # groups per core
B_PER_GROUP = B_PER_CORE // N_GROUPS  # 8 batches -> 128 partitions
P = 128
CHUNK = 128
N_CHUNKS = HW // CHUNK          # 32
NC80 = B_PER_GROUP * NUM_COLORS  # 80
GRID_COLS = N_GROUPS * N_CHUNKS * B_PER_GROUP  # 2048, col = g*256 + k*8 + b

USE_FP8 = True

_CACHE = {}


def _build_nc():
    import concourse.bacc as bacc
    import concourse.tile as tile
    import concourse.bass as bass
    from concourse import mybir

    f32 = mybir.dt.float32
    bf16 = mybir.dt.bfloat16
    attn_dt = mybir.dt.float8e4 if USE_FP8 else bf16
    OP = mybir.AluOpType
    AF = mybir.ActivationFunctionType

    nc = bacc.Bacc(
        "TRN2",
        target_bir_lowering=False,
        debug=False,
        num_devices=N_CORES,
        enable_partition_id=False,
    )

    attn_in = nc.dram_tensor(
        "attn_in", [N_GROUPS * P, HW], attn_dt, kind="ExternalInput"
    ).ap()
    # grid values 0..9 are exact in fp8e4; shipped as fp8 (halves the
    # transfer) and upcast to bf16 on the idle ACT engine for the 2x-mode
    # DVE is_equal
    grid_in = nc.dram_tensor(
        "grid_in", [P, GRID_COLS], mybir.dt.float8e4, kind="ExternalInput"
    ).ap()
    # raw 80-col histograms per group; host extracts the block diagonal
    cd_out = nc.dram_tensor(
        "cd_out", [P, N_GROUPS * NC80], bf16, kind="ExternalOutput"
    ).ap()

    with tile.TileContext(nc) as tc:
        with ExitStack() as ctx:
            singles = ctx.enter_context(tc.tile_pool(name="singles", bufs=1))
            psum = ctx.enter_context(
                tc.tile_pool(name="psum", bufs=4, space="PSUM")
            )

            # const_cb[p, c*8+b] = c (0..9, exact in bf16) -- needed by the
            # first mask op; Pool is free.
            const_cb = singles.tile([P, NC80], bf16)
            nc.gpsimd.iota(
                const_cb,
                pattern=[[1, NUM_COLORS], [0, B_PER_GROUP]],
                base=0,
                channel_multiplier=0,
                allow_small_or_imprecise_dtypes=True,
            )

            # Strictly ordered HWDGE DMAs on the SP ring. Host grid layout
            # col = g*256 + k*8 + b: the first half covers groups 0-3 and
            # gates mask0; the second half rides behind the first attn group.
            gridT8 = singles.tile([P, GRID_COLS], mybir.dt.float8e4)
            gridT = singles.tile([P, GRID_COLS], bf16)
            half = GRID_COLS // 2
            nc.sync.dma_start(out=gridT8[:, 0:half], in_=grid_in[:, 0:half])

            attn_sb = []
            for g in range(N_GROUPS):
                t = singles.tile([P, HW], attn_dt, name=f"attn{g}")
                if g == N_GROUPS - 1:
                    # split the last group so the PE tail trails the DMA
                    # stream by only half a group
                    hw2 = HW // 2
                    nc.sync.dma_start(
                        out=t[:, 0:hw2], in_=attn_in[g * P : (g + 1) * P, 0:hw2]
                    )
                    nc.sync.dma_start(
                        out=t[:, hw2:HW], in_=attn_in[g * P : (g + 1) * P, hw2:HW]
                    )
                else:
                    nc.sync.dma_start(out=t, in_=attn_in[g * P : (g + 1) * P, :])
                attn_sb.append(t)
                if g == 0:
                    nc.sync.dma_start(
                        out=gridT8[:, half:GRID_COLS],
                        in_=grid_in[:, half:GRID_COLS],
                    )

            # exact fp8 -> bf16 upcasts, per half, on ACT
            nc.scalar.activation(gridT[:, 0:half], gridT8[:, 0:half], AF.Copy)
            nc.scalar.activation(
                gridT[:, half:GRID_COLS], gridT8[:, half:GRID_COLS], AF.Copy
            )

            cdq_all = singles.tile([P, N_GROUPS * NC80], bf16)

            # ---- one-hot masks, one is_equal per group on DVE ----
            # mask_g flat [128, 2560]: col = k*80 + c*8 + b
            def build_mask(g):
                mq = singles.tile([P, N_CHUNKS * NC80], bf16, name=f"mask{g}")
                gT = gridT[:, :]
                in0 = bass.AP(
                    tensor=gT.tensor,
                    offset=gT.offset + g * (N_CHUNKS * B_PER_GROUP),
                    ap=[
                        gT.ap[0],
                        [B_PER_GROUP, N_CHUNKS],
                        [0, NUM_COLORS],
                        [1, B_PER_GROUP],
                    ],
                )
                cC = const_cb[:, :]
                in1 = bass.AP(
                    tensor=cC.tensor,
                    offset=cC.offset,
                    ap=[cC.ap[0], [0, N_CHUNKS], [1, NC80]],
                )
                mk = mq[:, :]
                mout = bass.AP(
                    tensor=mk.tensor,
                    offset=mk.offset,
                    ap=[mk.ap[0], [NC80, N_CHUNKS], [1, NC80]],
                )
                nc.vector.tensor_tensor(out=mout, in0=in0, in1=in1, op=OP.is_equal)
                return mq

            masks = [build_mask(0), build_mask(1)]

            for g in range(N_GROUPS):
                attn_bf = attn_sb[g]
                mask = masks[g]

                ps = psum.tile([P, 512], f32, name="ps", tag="ps")
                ps_c = ps[:, 0:NC80]
                for k in range(N_CHUNKS):
                    nc.tensor.matmul(
                        ps_c,
                        attn_bf[:, k * CHUNK : (k + 1) * CHUNK],
                        mask[:, k * NC80 : (k + 1) * NC80],
                        start=(k == 0),
                        stop=(k == N_CHUNKS - 1),
                    )

                if g + 2 < N_GROUPS:
                    masks.append(build_mask(g + 2))

                # the only post-op: PSUM -> SBUF bf16 on the idle ACT engine
                nc.scalar.activation(
                    cdq_all[:, g * NC80 : (g + 1) * NC80], ps_c, AF.Copy
                )
                if g == N_GROUPS - 2:
                    # ship groups 0-6 while group 7 computes; only the last
                    # 80 columns remain on the critical tail
                    cut = (N_GROUPS - 1) * NC80
                    nc.sync.dma_start(
                        out=cd_out[:, 0:cut], in_=cdq_all[:, 0:cut]
                    )

            cut = (N_GROUPS - 1) * NC80
            nc.sync.dma_start(
                out=cd_out[:, cut : N_GROUPS * NC80],
                in_=cdq_all[:, cut : N_GROUPS * NC80],
            )

    nc.compile()
    return nc


def _get_nc():
    if "nc" not in _CACHE:
        _CACHE["nc"] = _build_nc()
    return _CACHE["nc"]


def _make_in_maps(attn_weights, grids):
    import ml_dtypes

    attn_np_dt = ml_dtypes.float8_e4m3 if USE_FP8 else ml_dtypes.bfloat16
    attn = np.asarray(attn_weights, dtype=np.float32)   # [512,16,64,64]
    grid = np.asarray(grids)                            # [512,64,64]
    in_maps = []
    for c in range(N_CORES):
        lo = c * B_PER_CORE
        # (g,b,s,k,p) -> (g,p,k,b,s): row g*128+p, col k*128 + b*16 + s
        a = attn[lo : lo + B_PER_CORE].reshape(
            N_GROUPS, B_PER_GROUP, S, N_CHUNKS, CHUNK
        )
        a = np.ascontiguousarray(a.transpose(0, 4, 3, 1, 2)).reshape(
            N_GROUPS * CHUNK, HW
        )
        # (g,b,k,p) -> (p,g,k,b): col g*256 + k*8 + b
        g_ = grid[lo : lo + B_PER_CORE].reshape(
            N_GROUPS, B_PER_GROUP, N_CHUNKS, CHUNK
        )
        g_ = np.ascontiguousarray(g_.transpose(3, 0, 2, 1)).reshape(
            CHUNK, GRID_COLS
        )
        in_maps.append(
            {
                "attn_in": a.astype(attn_np_dt),
                "grid_in": g_.astype(np.float32).astype(ml_dtypes.float8_e4m3),
            }
        )
    return in_maps


def kernel(attn_weights: np.ndarray, grids: np.ndarray) -> np.ndarray:
    from concourse.bass_utils import run_bass_kernel_spmd

    nc = _get_nc()
    in_maps = _make_in_maps(attn_weights, grids)
    res = run_bass_kernel_spmd(nc, in_maps, core_ids=list(range(N_CORES)))

    # Host finale: pick the block diagonal (row p=(b,s) owns batch p//16
    # of its group), then entropy + mean in f64.
    b_idx = (np.arange(P) // S)[:, None, None, None]     # [128,1,1,1]
    total = 0.0
    for c in range(N_CORES):
        r = res.results[c]["cd_out"].astype(np.float64)  # [128, 640]
        r = r.reshape(P, N_GROUPS, NUM_COLORS, B_PER_GROUP)
        cd = np.take_along_axis(r, b_idx, axis=3)[..., 0]  # [128, 8, 10]
        s_ = cd.sum(-1) + EPS
        p_ = cd / s_[..., None]
        ent = -(p_ * np.log(p_ + EPS)).sum(-1)
        total += float(ent.sum())
    return np.float32(total / (B * S))
